# revision 2
# baseline (speedup 1.0000x reference)
"""Contrastive loss kernel for Trainium2 (8 NeuronCores, batch-parallel), fp16.

Problem (hardcoded):
  X: (32, 16384, 256) f32   pair embeddings, e_a = X[..., :128], e_b = X[..., 128:]
  y: (32, 128, 128)  i32    adjacency in {0, 1}
  out: (32, 16384)   f32    where(y==1, dist2, relu(1 - dist2))

Strategy:
  - Data-parallel over batch: 4 batches per core, no communication.
  - Host casts X f32 -> fp16 (empirical rel err 5.2e-4 vs the 2e-2 gate),
    halving HBM traffic to 32 MiB/core: ~94 us DMA floor at 358 GB/s.
  - Flat layout: the core's (4, 16384, 256) slice is viewed as
    [128 partitions, 512 pairs, 256 feat] so every DMA chunk is contiguous
    per partition (32 KiB descriptors) and dist2/y/out all share the same
    [partition, pair] layout - no transpose, no PSUM.
  - Per chunk: DVE tensor_sub (fp16 2x mode) -> ACT Square -> DVE in-place
    pairwise tree of tensor_adds over feat (each stage 2x mode; total cost
    ~ one 2x pass, vs tensor_reduce which has no fast mode and costs 2x
    more). Final tree stage writes f32. DVE ~69 us + ACT ~57 us, both
    under the ~94 us DMA span.
"""

from contextlib import ExitStack

import numpy as np

import concourse.bass as bass
import concourse.tile as tile
from concourse import bacc, mybir
from concourse.bass_utils import run_bass_kernel_spmd

F32 = mybir.dt.float32
F16 = mybir.dt.float16
I32 = mybir.dt.int32

B, P, D = 32, 16384, 256
H = D // 2  # 128
N_CORES = 8
BPC = B // N_CORES          # batches per core
PART = 128                  # SBUF partitions
RPP = BPC * P // PART       # pairs per partition per core (512)


DEFAULT_CHUNKS = (16, 32, 64, 64, 64, 64, 64, 64, 64, 16)


def build_program(chunk_sizes=DEFAULT_CHUNKS, xbufs=3, dma_split=True,
                  passes=1):
    """Per-core program. X arrives host-cast to fp16 in a [128, 512, 256]
    layout (partition-major flat view of the (4, 16384, 256) slice).

    chunk_sizes: pairs-per-partition per pipeline chunk (sums to 512).
    Small first chunk shortens pipeline fill; small last chunk shortens
    the drain (the tail compute after the final DMA lands).

    passes>1 repeats the whole computation (idempotent) - used only for
    marginal-time benchmarking, never for the graded kernel."""
    assert sum(chunk_sizes) == RPP

    nc = bacc.Bacc("TRN2", target_bir_lowering=False, debug=False,
                   num_devices=N_CORES)
    X = nc.dram_tensor("X", [PART, RPP, D], F16, kind="ExternalInput").ap()
    Y = nc.dram_tensor("y", [PART, RPP], I32, kind="ExternalInput").ap()
    O = nc.dram_tensor("out", [PART, RPP], F32, kind="ExternalOutput").ap()

    with tile.TileContext(nc) as tc, ExitStack() as ctx:
        xpool = ctx.enter_context(tc.tile_pool(name="x", bufs=xbufs))
        dpool = ctx.enter_context(tc.tile_pool(name="diff", bufs=2))
        qpool = ctx.enter_context(tc.tile_pool(name="sq", bufs=2))
        rpool = ctx.enter_context(tc.tile_pool(name="res", bufs=2))
        spool = ctx.enter_context(tc.tile_pool(name="small", bufs=4))

        for _ in range(passes):
            yt = spool.tile([PART, RPP], I32)
            nc.scalar.dma_start(yt[:], Y[:, :])

            res32 = rpool.tile([PART, RPP], F32)
            outt = spool.tile([PART, RPP], F32)
            off = 0
            for g, S in enumerate(chunk_sizes):
                sl = slice(off, off + S)
                off += S
                xt = xpool.tile([PART, S, D], F16)
                # Alternate HWDGE (SP ring) / SWDGE (gpsimd) so X loads
                # stream from two queues; ACT keeps only the squares.
                dma_eng = nc.gpsimd if (dma_split and g % 2) else nc.sync
                dma_eng.dma_start(xt[:], X[:, sl, :])

                dft = dpool.tile([PART, S, H], F16)
                nc.vector.tensor_sub(dft[:], xt[:, :, 0:H], xt[:, :, H:D])

                sq = qpool.tile([PART, S, H], F16)
                nc.scalar.activation(sq[:], dft[:],
                                     mybir.ActivationFunctionType.Square)

                # dist2 = sum over feat: in-place pairwise tree in sq.
                # fp16 partials round once per stage (7 stages); measured
                # end-to-end rel err ~1e-3 vs the 2e-2 tolerance.
                with nc.allow_low_precision(
                        "fp16 pairwise-tree dist2; measured ~1e-3 rel err "
                        "vs 2e-2 tolerance"):
                    m = H // 2
                    while m >= 2:
                        nc.vector.tensor_add(sq[:, :, 0:m], sq[:, :, 0:m],
                                             sq[:, :, m:2 * m])
                        m //= 2
                    nc.vector.tensor_add(res32[:, sl], sq[:, :, 0],
                                         sq[:, :, 1])

                # Tail per chunk: out = relu(1 - dist2), overwrite y==1
                # entries with dist2. Keeps the final out-DMA's dependency
                # to just the (small) last chunk.
                nc.scalar.activation(outt[:, sl], res32[:, sl],
                                     mybir.ActivationFunctionType.Relu,
                                     scale=-1.0, bias=1.0)
                nc.vector.copy_predicated(outt[:, sl], yt[:, sl],
                                          res32[:, sl])

            nc.sync.dma_start(O[:, :], outt[:])

    nc.compile()
    return nc


_PROGRAM_CACHE = {}


def _get_program():
    if "nc" not in _PROGRAM_CACHE:
        _PROGRAM_CACHE["nc"] = build_program()
    return _PROGRAM_CACHE["nc"]


def make_in_maps(X, y):
    """Host-side shard + cast: per-core fp16 [128, 512, 256] X view and
    [128, 512] i32 y view (both pure reshapes of the batch slice)."""
    X = np.asarray(X, dtype=np.float32)
    y = np.asarray(y, dtype=np.int32).reshape(B, P)
    assert X.shape == (B, P, D)
    X16 = X.astype(np.float16)
    return [
        {"X": X16[c * BPC:(c + 1) * BPC].reshape(PART, RPP, D),
         "y": y[c * BPC:(c + 1) * BPC].reshape(PART, RPP)}
        for c in range(N_CORES)
    ]


def kernel(X, y):
    import os
    if os.environ.get("BASS_TRACE"):
        # The axon NTFF trace path needs antenv.axon_hooks, which some
        # images lack; fall back to untraced execution rather than crash.
        try:
            import antenv.axon_hooks  # noqa: F401
        except ImportError:
            os.environ["BASS_NEVER_TRACE"] = "1"

    nc = _get_program()
    in_maps = make_in_maps(X, y)
    # The axon-tunneled devices occasionally come up wedged from a prior
    # session (NRT_EXEC_UNIT_UNRECOVERABLE); a backend reset + retry clears it.
    last_err = None
    for attempt in range(3):
        try:
            res = run_bass_kernel_spmd(nc, in_maps, list(range(N_CORES)))
            break
        except Exception as e:  # transient device/tunnel failures
            last_err = e
            import time

            import jax
            try:
                jax.clear_caches()
            except Exception:
                pass
            try:
                jax._src.api.clear_backends()
            except Exception:
                pass
            time.sleep(5.0 * (attempt + 1))
    else:
        raise last_err
    out = np.concatenate(
        [res.results[c]["out"].reshape(BPC, P) for c in range(N_CORES)], axis=0)
    return out.astype(np.float32)
